# revision 1
# baseline (speedup 1.0000x reference)
"""Bass/Trainium2 kernel for nn_Net_27882927686181 (gnn_message_passing), v2.

Computation: v0 = sigmoid(x + 1); 12 layers of
    v <- sigmoid(einsum('bmk,mk->bm', v[:, idx[l]], W[l]) + b[l])
with B=1024, M=2048, K=32, L=12.

Strategy (8 NeuronCores, SPMD, node-sharded):
  - Core c owns nodes [256c, 256c+256) of every layer, full batch 1024.
  - Layer value table vT [2048 rows, 1024 batch] f16 in Shared DRAM, row
    layout permuted to AllGather order: row(n) = 1024*blk + 128*core + p
    where blk=(n%256)//128, core=n//256, p=n%128.  Each layer does two
    AllGathers of [128, 1024] half-shards; the first overlaps the second
    block's compute.
  - Gathers via SWDGE dma_gather: 8 chunks/layer of 1024 int16 indices,
    each filling a [128, 8, 1024] SBUF tile (group g in [:, g, :], row
    p=32j+k of group g holds table[idx] for node 4g+j fan-in k).
  - PE: per 128-node block, 32 accumulating [128x128]x[128x512] f16
    matmuls per psum half; stationary weights are block-diagonal
    expansions built ON DEVICE from the compact W by DVE tensor_scalar
    multiplies against 0/1 masks (masks built once by strided memsets).
  - ACT applies sigmoid(psum + bias); result f16 is stored/AllGathered.

Inputs shipped per core (~1.4 MB): xs [128,2048] f16 (node shard of x),
idx16 [16,6144] int16, Wc [128,768] f16, bp [128,24] f32.  Output
[128,2048] f16 per core.
"""

import os
import numpy as np

B, M, K, L = 1024, 2048, 32, 12
N_CORES = 8
NPC = M // N_CORES          # 256 nodes per core
GROUPS = NPC // 4           # 64 groups of 4 nodes per layer
CHUNK_GROUPS = 8            # groups per dma_gather
CHUNKS = GROUPS // CHUNK_GROUPS       # 8 chunks per layer
CHUNK_IDXS = CHUNK_GROUPS * 128       # 1024 indices per gather
CHUNK_COLS = CHUNK_IDXS // 16         # 64 idx columns per chunk
IDX_COLS = L * CHUNKS * CHUNK_COLS    # 6144

_cache = {}


def _patch_walrus():
    """Extra walrus passes for register-operand instructions (dma_gather)."""
    import concourse.bass_utils as bu
    if getattr(bu, "_ant_dge_patched", False):
        return
    orig = bu.run_command
    dge = ("--dge-levels=io,spill_reload,scalar_dynamic_offset,"
           "vector_dynamic_offsets,dst_reduce,transpose")

    def patched(argv, **kwargs):
        if argv and "walrus_driver" in str(argv[0]):
            argv = list(argv)
            for i, a in enumerate(argv):
                if a == "--pass":
                    passes = argv[i + 1].split(",")
                    for p in ("expand_inst_late", "coloring_allocator_reg"):
                        if p not in passes:
                            passes.insert(passes.index("codegen"), p)
                    # birverifier runs before coloring_allocator_reg and
                    # rejects the not-yet-assigned register operands of
                    # dma_gather; drop it.
                    passes = [p for p in passes if p != "birverifier"]
                    argv[i + 1] = ",".join(passes)
                    break
            argv.append(dge)
        return orig(argv, **kwargs)

    bu.run_command = patched
    bu._ant_dge_patched = True


def _split_multi_waits(nc, max_waits=1):
    """walrus codegen rejects >max sem waits per instruction; split onto NOPs."""
    import bass_rust
    from concourse import mybir
    n = 0
    for f in nc.m.functions:
        for blk in f.blocks:
            il = blk.instructions
            i = 0
            while i < len(il):
                inst = il[i]
                si = inst.sync_info
                if si is not None and len(si.on_wait) > max_waits:
                    waits = list(si.on_wait)
                    si.on_wait = waits[:max_waits]
                    extra = waits[max_waits:]
                    pos = i
                    for j in range(0, len(extra), max_waits):
                        nop = mybir.InstNoOp(name=f"Wsplit{n}-{j}", ins=[], outs=[])
                        nop.engine = inst.engine
                        nop.sync_info = bass_rust.SyncInfo(
                            on_wait=extra[j:j + max_waits], on_update=[])
                        il.insert(pos, nop)
                        pos += 1
                        i += 1
                    n += 1
                i += 1
    return n


def _build():
    import concourse.bass as bass
    import concourse.tile as tile
    from concourse import mybir
    from concourse.tile import add_dep_helper

    _patch_walrus()

    f32 = mybir.dt.float32
    f16 = mybir.dt.float16
    i16 = mybir.dt.int16
    # defer register assignment to walrus's coloring_allocator_reg pass
    # (as Bacc does) — needed for dma_gather's num_idxs register operand
    bass.Bass._defer_register_allocation = True
    nc = bass.Bass("TRN2", target_bir_lowering=False, debug=False,
                   num_devices=N_CORES)

    xs_d = nc.dram_tensor("xs", [128, 2 * B], f16, kind="ExternalInput").ap()
    idx_d = nc.dram_tensor("idx16", [16, IDX_COLS], i16,
                           kind="ExternalInput").ap()
    wc_d = nc.dram_tensor("wc", [128, L * GROUPS], f16,
                          kind="ExternalInput").ap()
    b_d = nc.dram_tensor("bp", [128, L * 2], f32, kind="ExternalInput").ap()
    out_d = nc.dram_tensor("out", [128, 2 * B], f16,
                           kind="ExternalOutput").ap()

    # 3-deep table rotation for pipeline slack on the WAR edge.
    _vt_kw = {} if os.environ.get("K_NOSHARED") else dict(addr_space="Shared")
    vt = [nc.dram_tensor(f"vt{i}", [M, B], f16, **_vt_kw).ap()
          for i in range(3)]
    shard = [nc.dram_tensor(f"shard{i}", [128, B], f16).ap() for i in range(2)]

    with tile.TileContext(nc) as tc:
        with tc.tile_pool(name="const", bufs=1) as cpool, \
             tc.tile_pool(name="wst", bufs=3) as wpool, \
             tc.tile_pool(name="gath", bufs=6) as gpool, \
             tc.tile_pool(name="sig", bufs=2) as spool, \
             tc.tile_pool(name="psum", bufs=2, space="PSUM") as ppool:

            # ---- constants ----
            if not os.environ.get("K_NOLIB"):
                from concourse.library_config import mlp as _mlp_lib
                nc.gpsimd.load_library(_mlp_lib)

            # xs first: the init sigmoid -> AllGather chain is the critical
            # path at startup; the idx replication (8x192KB on the same
            # queue) would delay it by ~70us.
            xs_sb = cpool.tile([128, 2 * B], f16)
            nc.sync.dma_start(xs_sb[:], xs_d[:])

            idx_sb = cpool.tile([128, IDX_COLS], i16)
            idx_loads = []
            for r in range(8):  # replicate across the 8 gpsimd sub-cores
                ld = nc.sync.dma_start(idx_sb[16 * r:16 * (r + 1), :], idx_d[:])
                idx_loads.append(ld.ins)
            wc_sb = cpool.tile([128, L * GROUPS], f16)
            nc.sync.dma_start(wc_sb[:], wc_d[:])
            wc32 = cpool.tile([128, L * GROUPS], f32)
            nc.scalar.copy(wc32[:], wc_sb[:])
            b_sb = cpool.tile([128, L * 2], f32)
            nc.sync.dma_start(b_sb[:], b_d[:])

            # masks[p, gl*128 + c] = 1 if c == 4*gl + p//32 else 0
            masks = cpool.tile([128, 32 * 128], f16)
            nc.vector.memset(masks[:], 0.0)
            for j in range(4):
                nc.vector.memset(masks[32 * j:32 * (j + 1),
                                       j:j + 132 * 31 + 1:132], 1.0)

            # ---- init: table0 = sigmoid(x + 1), via 2 half AllGathers ----
            table_writers = []
            for blk in range(2):
                s0 = spool.tile([128, B], f16, tag="sig")
                nc.scalar.activation(s0[:], xs_sb[:, blk * B:(blk + 1) * B],
                                     mybir.ActivationFunctionType.Sigmoid,
                                     bias=1.0, scale=1.0)
                st = nc.sync.dma_start(shard[blk][:], s0[:])
                if os.environ.get("K_NOCOLL"):
                    cc = nc.sync.dma_start(
                        vt[0][1024 * blk:1024 * blk + 128, :], shard[blk][:])
                else:
                    cc = nc.gpsimd.collective_compute(
                        "AllGather", mybir.AluOpType.bypass,
                        replica_groups=[list(range(N_CORES))],
                        ins=[shard[blk][:]],
                        outs=[vt[0][1024 * blk:1024 * (blk + 1), :]])
                add_dep_helper(cc.ins, st.ins, sync=True, reason="shard stored")
                table_writers.append(cc.ins)

            gathers_by_layer = [None, None]  # layers l-1, l-2 gather insts

            for l in range(L):
                src = vt[l % 3]
                layer_gathers = []
                new_writers = []
                shard_stores = []

                for blk in range(2):
                    # stationary block-diag weights for this 128-node block
                    wst = wpool.tile([128, 32 * 128], f16, tag="w")
                    for gl in range(32):
                        col = l * GROUPS + blk * 32 + gl
                        nc.vector.tensor_scalar_mul(
                            wst[:, gl * 128:(gl + 1) * 128],
                            masks[:, gl * 128:(gl + 1) * 128],
                            wc32[:, col:col + 1])

                    psum = ppool.tile([128, B], f32, tag="ps")
                    for ci in range(4):
                        q = blk * 4 + ci
                        gt = gpool.tile([128, CHUNK_GROUPS, B], f16, tag="g")
                        c0 = (l * CHUNKS + q) * CHUNK_COLS
                        gi_inst = nc.gpsimd.dma_gather(
                            gt[:], src[:], idx_sb[:, c0:c0 + CHUNK_COLS],
                            CHUNK_IDXS, CHUNK_IDXS, B)
                        layer_gathers.append(gi_inst.ins)
                        for w in table_writers:
                            add_dep_helper(gi_inst.ins, w, sync=True,
                                           reason="table ready")
                        for g in range(CHUNK_GROUPS):
                            for h in range(2):
                                nc.tensor.matmul(
                                    out=psum[:, h * 512:(h + 1) * 512],
                                    lhsT=wst[:, (ci * 8 + g) * 128:
                                             (ci * 8 + g + 1) * 128],
                                    rhs=gt[:, g, h * 512:(h + 1) * 512],
                                    start=(ci == 0 and g == 0),
                                    stop=(ci == 3 and g == 7))

                    sig = spool.tile([128, B], f16, tag="sig")
                    nc.scalar.activation(sig[:], psum[:],
                                         mybir.ActivationFunctionType.Sigmoid,
                                         bias=b_sb[:, l * 2 + blk:l * 2 + blk + 1],
                                         scale=1.0)
                    if l == L - 1:
                        nc.sync.dma_start(out_d[:, blk * B:(blk + 1) * B],
                                          sig[:])
                    else:
                        shard_stores.append(
                            nc.sync.dma_start(shard[blk][:], sig[:]).ins)

                # Collectives AFTER both blocks' gathers in the gpsimd
                # stream: an AG trigger between the two gather batches
                # would stall blk1's descriptor generation behind blk0's
                # sigmoid+store.
                if l < L - 1:
                    dst = vt[(l + 1) % 3]
                    for blk in range(2):
                        cc = nc.gpsimd.collective_compute(
                            "AllGather", mybir.AluOpType.bypass,
                            replica_groups=[list(range(N_CORES))],
                            ins=[shard[blk][:]],
                            outs=[dst[1024 * blk:1024 * (blk + 1), :]])
                        add_dep_helper(cc.ins, shard_stores[blk], sync=True,
                                       reason="shard stored")
                        # WAR: dst was read by layer l-2's gather DMAs
                        if gathers_by_layer[1]:
                            for gi in gathers_by_layer[1]:
                                add_dep_helper(cc.ins, gi, sync=True,
                                               reason="table WAR")
                        new_writers.append(cc.ins)

                table_writers = new_writers
                gathers_by_layer = [layer_gathers, gathers_by_layer[0]]

    # Tile schedules each gen_mode==1 prep onto a DMASW{k} proc and points
    # consumer waits at its own DMASW{k}_<scope> semaphore, but the
    # descriptor-baked completion sem stays the one passed via sem=.  Swap
    # each prep's OnUpdate[0] to the proc sem Tile actually waits on.
    from concourse.tile_sem_assignment import PROC_NAME_TO_IDX
    idx_to_dmasw = {v: k for k, v in PROC_NAME_TO_IDX.items()
                    if k.startswith("DMASW")}
    sem_ids = {}
    for f in nc.m.functions:
        for blk in f.blocks:
            for i in blk.instructions:
                si = i.sync_info
                if si is None:
                    continue
                for w in list(si.on_wait) + list(si.on_update):
                    if w.sync_type == "semaphore":
                        sem_ids[w.ant_name] = w.id
    for f in nc.m.functions:
        for blk in f.blocks:
            for i in blk.instructions:
                if (isinstance(i, mybir.InstDMAGatherAnt)
                        and i.gen_mode == 1):
                    proc = i.bass_scheduled_proc
                    pname = idx_to_dmasw.get(proc)
                    assert pname is not None, (i.name, proc)
                    target = [n for n in sem_ids
                              if n.startswith(pname + "_")]
                    assert len(target) == 1, (pname, target)
                    si = i.sync_info
                    ups = list(si.on_update)
                    assert ups[0].ant_name == "gather_dma", ups
                    ups[0].ant_name = target[0]
                    ups[0].id = sem_ids[target[0]]
                    si.on_update = ups

    if os.environ.get("K_SIM"):
        return nc  # leave IR un-postprocessed for CoreSim
    import bass_rust
    bass_rust.alloc_regs(nc.main_func, list(nc.engines), nc.inst_map)
    # lower bass_isa InstISA subclasses (the library reload) to raw ISA
    mybir.codegen_inst_isa_subclasses(nc)
    # alloc_regs DCEs dead preamble reg-moves into bare InstEventSemaphore
    # placeholders; walrus codegen rejects those without sync info.
    for f in nc.m.functions:
        for blk in f.blocks:
            blk.instructions = [
                i for i in blk.instructions
                if not (isinstance(i, mybir.InstEventSemaphore)
                        and i.sync_info is None)]
    _split_multi_waits(nc, max_waits=1)
    return nc


def _get_runner():
    if "runner" in _cache:
        return _cache["runner"]
    import jax
    import concourse.mybir as mybir
    import concourse.bass2jax as bass2jax
    from concourse.bass2jax import _bass_exec_p, install_neuronx_cc_hook
    from jax.sharding import Mesh, PartitionSpec
    from jax.experimental.shard_map import shard_map

    nc = _build()
    install_neuronx_cc_hook()

    partition_name = nc.partition_id_tensor.name if nc.partition_id_tensor else None
    in_names, out_names, out_avals, zero_outs = [], [], [], []
    for alloc in nc.m.functions[0].allocations:
        if not isinstance(alloc, mybir.MemoryLocationSet):
            continue
        name = alloc.memorylocations[0].name
        if alloc.kind == "ExternalInput":
            if name != partition_name:
                in_names.append(name)
        elif alloc.kind == "ExternalOutput":
            shape = tuple(alloc.tensor_shape)
            dtype = mybir.dt.np(alloc.dtype)
            out_names.append(name)
            out_avals.append(jax.core.ShapedArray(shape, dtype))
            zero_outs.append(np.zeros(shape, dtype))
    n_params = len(in_names)
    all_in = in_names + out_names
    if partition_name is not None:
        all_in.append(partition_name)

    def _body(*args):
        operands = list(args)
        if partition_name is not None:
            operands.append(bass2jax.partition_id_tensor())
        return tuple(_bass_exec_p.bind(
            *operands,
            out_avals=tuple(out_avals),
            in_names=tuple(all_in),
            out_names=tuple(out_names),
            lowering_input_output_aliases=(),
            sim_require_finite=True,
            sim_require_nnan=True,
            nc=nc))

    devices = jax.devices()[:N_CORES]
    mesh = Mesh(np.asarray(devices), ("core",))
    in_specs = (PartitionSpec("core"),) * (n_params + len(out_names))
    out_specs = (PartitionSpec("core"),) * len(out_names)
    f = jax.jit(shard_map(_body, mesh=mesh, in_specs=in_specs,
                          out_specs=out_specs, check_rep=False),
                keep_unused=True)
    _cache["runner"] = (f, in_names, out_names, zero_outs)
    return _cache["runner"]


def _prep_inputs(x, idx, W, b):
    """Host-side layout prep (sharding + permutation only)."""
    # xs[c][p, blk*B + b] = x[b, 256c + 128blk + p]
    xs = np.ascontiguousarray(
        x.T.reshape(N_CORES, 2, 128, B).transpose(0, 2, 1, 3)
        .reshape(N_CORES, 128, 2 * B)).astype(np.float16)

    # permuted table row of source node n: 1024*((n%256)//128) + 128*(n//256) + n%128
    n = idx.astype(np.int64)
    r = 1024 * ((n % 256) // 128) + 128 * (n // 256) + (n % 128)
    # value[l, c, q, gi, j, k] with node = 256c + 4(8q+gi) + j
    v = r.reshape(L, N_CORES, CHUNKS, CHUNK_GROUPS, 4, K)
    # flatten i = gi*128 + 32j + k; X[c][i%16, (l*CHUNKS+q)*64 + i//16]
    v = v.transpose(1, 0, 2, 3, 4, 5).reshape(N_CORES, L * CHUNKS, CHUNK_IDXS)
    v = v.reshape(N_CORES, L * CHUNKS, CHUNK_COLS, 16)
    idx16 = np.ascontiguousarray(
        v.transpose(0, 3, 1, 2).reshape(N_CORES, 16, IDX_COLS)).astype(np.int16)

    # Wc[c][32j+k, l*64+g] = W[l, 256c+4g+j, k]
    Wr = W.reshape(L, N_CORES, GROUPS, 4, K)
    wc = np.ascontiguousarray(
        Wr.transpose(1, 3, 4, 0, 2).reshape(N_CORES, 128, L * GROUPS)
    ).astype(np.float16)

    # bp[c][p, l*2+blk] = b[l, 256c+128blk+p]
    br = b.reshape(L, N_CORES, 2, 128)
    bp = np.ascontiguousarray(
        br.transpose(1, 3, 0, 2).reshape(N_CORES, 128, L * 2)).astype(np.float32)

    return [{"xs": xs[c], "idx16": idx16[c], "wc": wc[c], "bp": bp[c]}
            for c in range(N_CORES)]


def device_args(per_core, in_names, zero_outs):
    """device_put per-core inputs with the matching mesh sharding so calls
    don't trigger a per-call resharding (jit__multi_slice) on device."""
    import jax
    from jax.sharding import Mesh, NamedSharding, PartitionSpec
    mesh = Mesh(np.asarray(jax.devices()[:N_CORES]), ("core",))
    sh = NamedSharding(mesh, PartitionSpec("core"))
    args = []
    for n in in_names:
        cat = np.concatenate([per_core[c][n] for c in range(N_CORES)], axis=0)
        args.append(jax.device_put(cat, sh))
    for z in zero_outs:
        args.append(jax.device_put(np.concatenate([z] * N_CORES, axis=0), sh))
    return args


def kernel(x, idx, W, b):
    import jax
    x = np.asarray(x, dtype=np.float32)
    idx = np.asarray(idx, dtype=np.int32)
    W = np.asarray(W, dtype=np.float32)
    b = np.asarray(b, dtype=np.float32)

    f, in_names, out_names, zero_outs = _get_runner()
    per_core = _prep_inputs(x, idx, W, b)
    args = device_args(per_core, in_names, zero_outs)

    outs = f(*args)
    jax.block_until_ready(outs)
    full = np.asarray(outs[0])                       # [8*128, 2*B] f16
    # out[b, 256c+128blk+p] = full[128c+p, blk*B+b]
    res = full.reshape(N_CORES, 128, 2, B).transpose(3, 0, 2, 1)
    return np.ascontiguousarray(res.reshape(B, M)).astype(np.float32)


if __name__ == "__main__":
    rng = np.random.default_rng(0)
    x = rng.standard_normal((B, M)).astype(np.float32)
    idx = rng.integers(0, M, size=(L, M, K)).astype(np.int32)
    W = rng.standard_normal((L, M, K)).astype(np.float32)
    b = rng.standard_normal((L, M)).astype(np.float32)
    out = kernel(x=x, idx=idx, W=W, b=b)
    v = 1.0 / (1.0 + np.exp(-(x + 1.0)))
    for l in range(L):
        g = v[:, idx[l]]
        v = 1.0 / (1.0 + np.exp(-(np.einsum('bmk,mk->bm', g, W[l]) + b[l])))
    err = np.abs(out - v).max() / max(np.abs(v).max(), 1e-9)
    print("rel err vs numpy:", err)



# revision 2
# speedup vs baseline: 16.3534x; 16.3534x over previous
"""Bass/Trainium2 kernel for nn_Net_27882927686181 (gnn_message_passing), v4.

Computation: v0 = sigmoid(x + 1); 12 layers of
    v <- sigmoid(einsum('bmk,mk->bm', v[:, idx[l]], W[l]) + b[l])
with B=1024, M=2048, K=32, L=12.

Strategy (8 NeuronCores, SPMD, batch-sharded, ZERO collectives):
  - The gather+einsum is reformulated as a dense matmul with the scatter
    matrix S_l[in_node, out_node] = sum_k W[l, out, k]*[idx[l,out,k]==in],
    built on the host:  v_{l+1} = sigmoid(S_l^T v_l + b_l).
  - Core c owns batch rows [128c, 128c+128); v lives entirely in SBUF as
    [128 nodes (partition), 16 node-blocks x 128 batch] f16 tiles, so the
    layout is identical between layers (out partition = node): no
    transpose, no inter-core exchange.  (AllGather costs ~30us/call in
    this environment, which kills every node-sharded alternative.)
  - Per layer: 16 out-blocks x 16 in-chunks accumulating [128x128]x
    [128x128] f16 matmuls; ACT applies sigmoid(psum + b) with
    per-partition bias.
  - S (100 MB f16, replicated on every core) streams from DRAM in 2 MB
    double-buffered tiles; the kernel runs at the PE/DMA roofline
    (~320-335us/exec vs ~3ms for the SWDGE-gather formulation).
  - `rep` builds a hardware For_i loop that executes the whole network
    rep times back-to-back on device (bit-identical output), letting the
    harness amortize the ~80ms axon dispatch round-trip when timing.

Inputs per core: xs [128,2048] f16 (batch shard of x^T), sc [128, L*32768]
f16 (replicated S stream), bp [128, L*16] f32 (replicated bias).
Output [128, 2048] f16 per core (node-major blocks x batch).
"""

import os
import numpy as np

B, M, K, L = 1024, 2048, 32, 12
N_CORES = 8
BC = B // N_CORES           # 128 batch rows per core
NB = M // 128               # 16 node blocks
SQ = 4                      # S dma chunks per layer (4 out-blocks each)
SCOLS = NB * NB * 128       # 32768 S cols per layer

_cache = {}


def _patch_walrus():
    """Extra walrus passes for register-operand instructions."""
    import concourse.bass_utils as bu
    if getattr(bu, "_ant_dge_patched", False):
        return
    orig = bu.run_command
    dge = ("--dge-levels=io,spill_reload,scalar_dynamic_offset,"
           "vector_dynamic_offsets,dst_reduce,transpose")

    def patched(argv, **kwargs):
        if argv and "walrus_driver" in str(argv[0]):
            argv = list(argv)
            for i, a in enumerate(argv):
                if a == "--pass":
                    passes = argv[i + 1].split(",")
                    for p in ("expand_inst_late", "coloring_allocator_reg"):
                        if p not in passes:
                            passes.insert(passes.index("codegen"), p)
                    passes = [p for p in passes if p != "birverifier"]
                    argv[i + 1] = ",".join(passes)
                    break
            argv.append(dge)
        return orig(argv, **kwargs)

    bu.run_command = patched
    bu._ant_dge_patched = True


def _split_multi_waits(nc, max_waits=1):
    """walrus codegen rejects >max sem waits per instruction; split onto NOPs."""
    import bass_rust
    from concourse import mybir
    n = 0
    for f in nc.m.functions:
        for blk in f.blocks:
            il = blk.instructions
            i = 0
            while i < len(il):
                inst = il[i]
                si = inst.sync_info
                if si is not None and len(si.on_wait) > max_waits:
                    waits = list(si.on_wait)
                    si.on_wait = waits[:max_waits]
                    extra = waits[max_waits:]
                    pos = i
                    for j in range(0, len(extra), max_waits):
                        nop = mybir.InstNoOp(name=f"Wsplit{n}-{j}", ins=[], outs=[])
                        nop.engine = inst.engine
                        nop.sync_info = bass_rust.SyncInfo(
                            on_wait=extra[j:j + max_waits], on_update=[])
                        il.insert(pos, nop)
                        pos += 1
                        i += 1
                    n += 1
                i += 1
    return n


def _build(rep=1):
    import contextlib
    import concourse.bass as bass
    import concourse.tile as tile
    from concourse import mybir

    _patch_walrus()

    f32 = mybir.dt.float32
    f16 = mybir.dt.float16
    bass.Bass._defer_register_allocation = True
    nc = bass.Bass("TRN2", target_bir_lowering=False, debug=False,
                   num_devices=N_CORES)

    xs_d = nc.dram_tensor("xs", [128, M], f16, kind="ExternalInput").ap()
    sc_d = nc.dram_tensor("sc", [128, L * SCOLS], f16,
                          kind="ExternalInput").ap()
    bp_d = nc.dram_tensor("bp", [128, L * NB], f32, kind="ExternalInput").ap()
    out_d = nc.dram_tensor("out", [128, M], f16, kind="ExternalOutput").ap()

    with tile.TileContext(nc) as tc:
        with tc.tile_pool(name="const", bufs=1) as cpool, \
             tc.tile_pool(name="sstream", bufs=2 * SQ) as spool, \
             tc.tile_pool(name="vbuf", bufs=2) as vpool, \
             tc.tile_pool(name="psum", bufs=6, space="PSUM") as ppool:

            loop = tc.For_i(0, rep) if rep > 1 else contextlib.nullcontext()
            with loop:
                bp_sb = cpool.tile([128, L * NB], f32)
                nc.sync.dma_start(bp_sb[:], bp_d[:])
                xs_sb = cpool.tile([128, M], f16)
                nc.sync.dma_start(xs_sb[:], xs_d[:])

                v_cur = vpool.tile([128, M], f16, tag="v")
                nc.scalar.activation(v_cur[:], xs_sb[:],
                                     mybir.ActivationFunctionType.Sigmoid,
                                     bias=1.0, scale=1.0)

                qcols = SCOLS // SQ  # 8192 cols per S chunk (4 out-blocks)
                for l in range(L):
                    s_tiles = []
                    for q in range(SQ):
                        st = spool.tile([128, qcols], f16, tag="s")
                        nc.sync.dma_start(
                            st[:], sc_d[:, l * SCOLS + q * qcols:
                                        l * SCOLS + (q + 1) * qcols])
                        s_tiles.append(st)

                    v_next = vpool.tile([128, M], f16, tag="v")
                    for o in range(NB):
                        st = s_tiles[o // 4]
                        base = (o % 4) * NB * 128
                        ps = ppool.tile([128, 128], f32, tag="ps")
                        for i in range(NB):
                            nc.tensor.matmul(
                                out=ps[:],
                                lhsT=st[:, base + i * 128: base + (i + 1) * 128],
                                rhs=v_cur[:, i * 128:(i + 1) * 128],
                                start=(i == 0), stop=(i == NB - 1))
                        nc.scalar.activation(
                            v_next[:, o * 128:(o + 1) * 128], ps[:],
                            mybir.ActivationFunctionType.Sigmoid,
                            bias=bp_sb[:, l * NB + o: l * NB + o + 1],
                            scale=1.0)
                    v_cur = v_next

                nc.sync.dma_start(out_d[:], v_cur[:])

    if os.environ.get("K_SIM"):
        return nc
    import bass_rust
    bass_rust.alloc_regs(nc.main_func, list(nc.engines), nc.inst_map)
    mybir.codegen_inst_isa_subclasses(nc)
    for f in nc.m.functions:
        for blk in f.blocks:
            blk.instructions = [
                i for i in blk.instructions
                if not (isinstance(i, mybir.InstEventSemaphore)
                        and i.sync_info is None)]
    _split_multi_waits(nc, max_waits=1)
    return nc


REPLICATED = {"sc", "bp"}


def _get_runner(rep=1):
    if ("runner", rep) in _cache:
        return _cache[("runner", rep)]
    import jax
    import concourse.mybir as mybir
    import concourse.bass2jax as bass2jax
    from concourse.bass2jax import _bass_exec_p, install_neuronx_cc_hook
    from jax.sharding import Mesh, PartitionSpec
    from jax.experimental.shard_map import shard_map

    nc = _build(rep)
    install_neuronx_cc_hook()

    partition_name = nc.partition_id_tensor.name if nc.partition_id_tensor else None
    in_names, out_names, out_avals, zero_outs = [], [], [], []
    for alloc in nc.m.functions[0].allocations:
        if not isinstance(alloc, mybir.MemoryLocationSet):
            continue
        name = alloc.memorylocations[0].name
        if alloc.kind == "ExternalInput":
            if name != partition_name:
                in_names.append(name)
        elif alloc.kind == "ExternalOutput":
            shape = tuple(alloc.tensor_shape)
            dtype = mybir.dt.np(alloc.dtype)
            out_names.append(name)
            out_avals.append(jax.core.ShapedArray(shape, dtype))
            zero_outs.append(np.zeros(shape, dtype))
    n_params = len(in_names)
    all_in = in_names + out_names
    if partition_name is not None:
        all_in.append(partition_name)

    def _body(*args):
        operands = list(args)
        if partition_name is not None:
            operands.append(bass2jax.partition_id_tensor())
        return tuple(_bass_exec_p.bind(
            *operands,
            out_avals=tuple(out_avals),
            in_names=tuple(all_in),
            out_names=tuple(out_names),
            lowering_input_output_aliases=(),
            sim_require_finite=True,
            sim_require_nnan=True,
            nc=nc))

    devices = jax.devices()[:N_CORES]
    mesh = Mesh(np.asarray(devices), ("core",))
    in_specs = tuple(
        PartitionSpec() if n in REPLICATED else PartitionSpec("core")
        for n in in_names) + (PartitionSpec("core"),) * len(out_names)
    out_specs = (PartitionSpec("core"),) * len(out_names)
    f = jax.jit(shard_map(_body, mesh=mesh, in_specs=in_specs,
                          out_specs=out_specs, check_rep=False),
                keep_unused=True)
    _cache[("runner", rep)] = (f, in_names, out_names, zero_outs)
    return _cache[("runner", rep)]


def _prep_inputs(x, idx, W, b):
    """Host-side layout prep: batch-shard x, scatter W into dense S."""
    # xs[c][p, 128*i + bb] = x[128c+bb, 128i+p]
    xT = np.ascontiguousarray(x.T)  # [node, batch]
    xs = np.ascontiguousarray(
        xT.reshape(NB, 128, N_CORES, BC).transpose(2, 1, 0, 3)
        .reshape(N_CORES, 128, M)).astype(np.float16)

    # S[l, in_node, out_node] = sum_k W[l, out, k] [idx[l, out, k] == in]
    S = np.zeros((L, M, M), np.float32)
    out_idx = np.broadcast_to(np.arange(M)[None, :, None], (L, M, K))
    lay_idx = np.broadcast_to(np.arange(L)[:, None, None], (L, M, K))
    np.add.at(S, (lay_idx.ravel(), idx.ravel(), out_idx.ravel()), W.ravel())
    # sc[p, ((l*NB + o)*NB + i)*128 + m] = S[l, 128i+p, 128o+m]
    sc = np.ascontiguousarray(
        S.reshape(L, NB, 128, NB, 128).transpose(2, 0, 3, 1, 4)
        .reshape(128, L * SCOLS)).astype(np.float16)

    # bp[m, l*NB + o] = b[l, 128o + m]
    bp = np.ascontiguousarray(
        b.reshape(L, NB, 128).transpose(2, 0, 1).reshape(128, L * NB)
    ).astype(np.float32)

    return xs, sc, bp


def device_args(xs, sc, bp, in_names, zero_outs):
    import jax
    from jax.sharding import Mesh, NamedSharding, PartitionSpec
    mesh = Mesh(np.asarray(jax.devices()[:N_CORES]), ("core",))
    sh_core = NamedSharding(mesh, PartitionSpec("core"))
    sh_rep = NamedSharding(mesh, PartitionSpec())
    per_name = {"xs": (xs.reshape(N_CORES * 128, M), sh_core),
                "sc": (sc, sh_rep), "bp": (bp, sh_rep)}
    args = []
    for n in in_names:
        arr, sh = per_name[n]
        args.append(jax.device_put(arr, sh))
    for z in zero_outs:
        args.append(jax.device_put(
            np.concatenate([z] * N_CORES, axis=0), sh_core))
    return args


def kernel(x, idx, W, b):
    import jax
    x = np.asarray(x, dtype=np.float32)
    idx = np.asarray(idx, dtype=np.int32)
    W = np.asarray(W, dtype=np.float32)
    b = np.asarray(b, dtype=np.float32)

    f, in_names, out_names, zero_outs = _get_runner()
    xs, sc, bp = _prep_inputs(x, idx, W, b)
    args = device_args(xs, sc, bp, in_names, zero_outs)

    outs = f(*args)
    jax.block_until_ready(outs)
    full = np.asarray(outs[0])                       # [8*128, 2048] f16
    # out[128c+bb, 128o+p] = full[c][p, 128o+bb]
    res = full.reshape(N_CORES, 128, NB, BC).transpose(0, 3, 2, 1)
    return np.ascontiguousarray(res.reshape(B, M)).astype(np.float32)


if __name__ == "__main__":
    rng = np.random.default_rng(0)
    x = rng.standard_normal((B, M)).astype(np.float32)
    idx = rng.integers(0, M, size=(L, M, K)).astype(np.int32)
    W = rng.standard_normal((L, M, K)).astype(np.float32)
    b = rng.standard_normal((L, M)).astype(np.float32)
    out = kernel(x=x, idx=idx, W=W, b=b)
    v = 1.0 / (1.0 + np.exp(-(x + 1.0)))
    for l in range(L):
        g = v[:, idx[l]]
        v = 1.0 / (1.0 + np.exp(-(np.einsum('bmk,mk->bm', g, W[l]) + b[l])))
    err = np.abs(out - v).max() / max(np.abs(v).max(), 1e-9)
    print("rel err vs numpy:", err)


# revision 3
# speedup vs baseline: 16.6206x; 1.0163x over previous
"""Bass/Trainium2 kernel for nn_Net_27882927686181 (gnn_message_passing), v4.

Computation: v0 = sigmoid(x + 1); 12 layers of
    v <- sigmoid(einsum('bmk,mk->bm', v[:, idx[l]], W[l]) + b[l])
with B=1024, M=2048, K=32, L=12.

Strategy (8 NeuronCores, SPMD, batch-sharded, ZERO collectives):
  - The gather+einsum is reformulated as a dense matmul with the scatter
    matrix S_l[in_node, out_node] = sum_k W[l, out, k]*[idx[l,out,k]==in],
    built on the host:  v_{l+1} = sigmoid(S_l^T v_l + b_l).
  - Core c owns batch rows [128c, 128c+128); v lives entirely in SBUF as
    [128 nodes (partition), 16 node-blocks x 128 batch] f16 tiles, so the
    layout is identical between layers (out partition = node): no
    transpose, no inter-core exchange.  (AllGather costs ~30us/call in
    this environment, which kills every node-sharded alternative.)
  - Per layer: 16 out-blocks x 16 in-chunks accumulating [128x128]x
    [128x128] f16 matmuls; ACT applies sigmoid(psum + b) with
    per-partition bias.
  - S (100 MB f16, replicated on every core) streams from DRAM in 2 MB
    double-buffered tiles; the kernel runs at the PE/DMA roofline
    (~320-335us/exec vs ~3ms for the SWDGE-gather formulation).
  - `rep` builds a hardware For_i loop that executes the whole network
    rep times back-to-back on device (bit-identical output), letting the
    harness amortize the ~80ms axon dispatch round-trip when timing.

Inputs per core: xs [128,2048] f16 (batch shard of x^T), sc [128, L*32768]
f16 (replicated S stream), bp [128, L*16] f32 (replicated bias).
Output [128, 2048] f16 per core (node-major blocks x batch).
"""

import os
import numpy as np

B, M, K, L = 1024, 2048, 32, 12
N_CORES = 8
BC = B // N_CORES           # 128 batch rows per core
NB = M // 128               # 16 node blocks
SQ = 4                      # S dma chunks per layer (4 out-blocks each)
SCOLS = NB * NB * 128       # 32768 S cols per layer

_cache = {}


def _patch_walrus():
    """Extra walrus passes for register-operand instructions."""
    import concourse.bass_utils as bu
    if getattr(bu, "_ant_dge_patched", False):
        return
    orig = bu.run_command
    dge = ("--dge-levels=io,spill_reload,scalar_dynamic_offset,"
           "vector_dynamic_offsets,dst_reduce,transpose")

    def patched(argv, **kwargs):
        if argv and "walrus_driver" in str(argv[0]):
            argv = list(argv)
            for i, a in enumerate(argv):
                if a == "--pass":
                    passes = argv[i + 1].split(",")
                    for p in ("expand_inst_late", "coloring_allocator_reg"):
                        if p not in passes:
                            passes.insert(passes.index("codegen"), p)
                    passes = [p for p in passes if p != "birverifier"]
                    argv[i + 1] = ",".join(passes)
                    break
            argv.append(dge)
        return orig(argv, **kwargs)

    bu.run_command = patched
    bu._ant_dge_patched = True


def _split_multi_waits(nc, max_waits=1):
    """walrus codegen rejects >max sem waits per instruction; split onto NOPs."""
    import bass_rust
    from concourse import mybir
    n = 0
    for f in nc.m.functions:
        for blk in f.blocks:
            il = blk.instructions
            i = 0
            while i < len(il):
                inst = il[i]
                si = inst.sync_info
                if si is not None and len(si.on_wait) > max_waits:
                    waits = list(si.on_wait)
                    si.on_wait = waits[:max_waits]
                    extra = waits[max_waits:]
                    pos = i
                    for j in range(0, len(extra), max_waits):
                        nop = mybir.InstNoOp(name=f"Wsplit{n}-{j}", ins=[], outs=[])
                        nop.engine = inst.engine
                        nop.sync_info = bass_rust.SyncInfo(
                            on_wait=extra[j:j + max_waits], on_update=[])
                        il.insert(pos, nop)
                        pos += 1
                        i += 1
                    n += 1
                i += 1
    return n


def _build(rep=1):
    import contextlib
    import concourse.bass as bass
    import concourse.tile as tile
    from concourse import mybir

    _patch_walrus()

    f32 = mybir.dt.float32
    f16 = mybir.dt.float16
    bass.Bass._defer_register_allocation = True
    nc = bass.Bass("TRN2", target_bir_lowering=False, debug=False,
                   num_devices=N_CORES)

    xs_d = nc.dram_tensor("xs", [128, M], f16, kind="ExternalInput").ap()
    sc_d = nc.dram_tensor("sc", [128, L * SCOLS], f16,
                          kind="ExternalInput").ap()
    bp_d = nc.dram_tensor("bp", [128, L * NB], f32, kind="ExternalInput").ap()
    out_d = nc.dram_tensor("out", [128, M], f16, kind="ExternalOutput").ap()

    with tile.TileContext(nc) as tc:
        with tc.tile_pool(name="const", bufs=1) as cpool, \
             tc.tile_pool(name="sstream", bufs=2 * SQ) as spool, \
             tc.tile_pool(name="vbuf", bufs=2) as vpool, \
             tc.tile_pool(name="psum", bufs=6, space="PSUM") as ppool:

            n_fill = int(os.environ.get("K_FILL", "48"))
            loop = tc.For_i(0, rep) if rep > 1 else contextlib.nullcontext()
            with loop:
                bp_sb = cpool.tile([128, L * NB], f32)
                nc.sync.dma_start(bp_sb[:], bp_d[:])
                xs_sb = cpool.tile([128, M], f16)
                nc.sync.dma_start(xs_sb[:], xs_d[:])

                v_cur = vpool.tile([128, M], f16, tag="v")
                nc.scalar.activation(v_cur[:], xs_sb[:],
                                     mybir.ActivationFunctionType.Sigmoid,
                                     bias=1.0, scale=1.0)

                qcols = SCOLS // SQ  # 8192 cols per S chunk (4 out-blocks)
                for l in range(L):
                    s_tiles = []
                    for q in range(SQ):
                        st = spool.tile([128, qcols], f16, tag="s")
                        nc.sync.dma_start(
                            st[:], sc_d[:, l * SCOLS + q * qcols:
                                        l * SCOLS + (q + 1) * qcols])
                        s_tiles.append(st)

                    v_next = vpool.tile([128, M], f16, tag="v")
                    for o in range(NB):
                        st = s_tiles[o // 4]
                        base = (o % 4) * NB * 128
                        ps = ppool.tile([128, 128], f32, tag="ps")
                        for i in range(NB):
                            nc.tensor.matmul(
                                out=ps[:],
                                lhsT=st[:, base + i * 128: base + (i + 1) * 128],
                                rhs=v_cur[:, i * 128:(i + 1) * 128],
                                start=(i == 0), stop=(i == NB - 1))
                        nc.scalar.activation(
                            v_next[:, o * 128:(o + 1) * 128], ps[:],
                            mybir.ActivationFunctionType.Sigmoid,
                            bias=bp_sb[:, l * NB + o: l * NB + o + 1],
                            scale=1.0)
                    v_cur = v_next

                    # p-state keepers: after each layer, dependency-free
                    # matmuls into a dead psum bank add just enough tensor-
                    # engine work that PE at the warm clock stays strictly
                    # slower than the S stream, so it never stalls waiting on
                    # DMA and never drops back to the 1.2 GHz p-state.
                    if n_fill and l < L - 1:
                        fps = ppool.tile([128, 512], f32, tag="fill", bufs=1)
                        for j in range(n_fill):
                            nc.tensor.matmul(
                                out=fps[:],
                                lhsT=s_tiles[0][:, :128],
                                rhs=s_tiles[0][:, 128:640],
                                start=True, stop=True)

                nc.sync.dma_start(out_d[:], v_cur[:])

    if os.environ.get("K_SIM"):
        return nc
    import bass_rust
    bass_rust.alloc_regs(nc.main_func, list(nc.engines), nc.inst_map)
    mybir.codegen_inst_isa_subclasses(nc)
    for f in nc.m.functions:
        for blk in f.blocks:
            blk.instructions = [
                i for i in blk.instructions
                if not (isinstance(i, mybir.InstEventSemaphore)
                        and i.sync_info is None)]
    _split_multi_waits(nc, max_waits=1)
    return nc


REPLICATED = {"sc", "bp"}


def _get_runner(rep=1):
    if ("runner", rep) in _cache:
        return _cache[("runner", rep)]
    import jax
    import concourse.mybir as mybir
    import concourse.bass2jax as bass2jax
    from concourse.bass2jax import _bass_exec_p, install_neuronx_cc_hook
    from jax.sharding import Mesh, PartitionSpec
    from jax.experimental.shard_map import shard_map

    nc = _build(rep)
    install_neuronx_cc_hook()

    partition_name = nc.partition_id_tensor.name if nc.partition_id_tensor else None
    in_names, out_names, out_avals, zero_outs = [], [], [], []
    for alloc in nc.m.functions[0].allocations:
        if not isinstance(alloc, mybir.MemoryLocationSet):
            continue
        name = alloc.memorylocations[0].name
        if alloc.kind == "ExternalInput":
            if name != partition_name:
                in_names.append(name)
        elif alloc.kind == "ExternalOutput":
            shape = tuple(alloc.tensor_shape)
            dtype = mybir.dt.np(alloc.dtype)
            out_names.append(name)
            out_avals.append(jax.core.ShapedArray(shape, dtype))
            zero_outs.append(np.zeros(shape, dtype))
    n_params = len(in_names)
    all_in = in_names + out_names
    if partition_name is not None:
        all_in.append(partition_name)

    def _body(*args):
        operands = list(args)
        if partition_name is not None:
            operands.append(bass2jax.partition_id_tensor())
        return tuple(_bass_exec_p.bind(
            *operands,
            out_avals=tuple(out_avals),
            in_names=tuple(all_in),
            out_names=tuple(out_names),
            lowering_input_output_aliases=(),
            sim_require_finite=True,
            sim_require_nnan=True,
            nc=nc))

    devices = jax.devices()[:N_CORES]
    mesh = Mesh(np.asarray(devices), ("core",))
    in_specs = tuple(
        PartitionSpec() if n in REPLICATED else PartitionSpec("core")
        for n in in_names) + (PartitionSpec("core"),) * len(out_names)
    out_specs = (PartitionSpec("core"),) * len(out_names)
    f = jax.jit(shard_map(_body, mesh=mesh, in_specs=in_specs,
                          out_specs=out_specs, check_rep=False),
                keep_unused=True)
    _cache[("runner", rep)] = (f, in_names, out_names, zero_outs)
    return _cache[("runner", rep)]


def _prep_inputs(x, idx, W, b):
    """Host-side layout prep: batch-shard x, scatter W into dense S."""
    # xs[c][p, 128*i + bb] = x[128c+bb, 128i+p]
    xT = np.ascontiguousarray(x.T)  # [node, batch]
    xs = np.ascontiguousarray(
        xT.reshape(NB, 128, N_CORES, BC).transpose(2, 1, 0, 3)
        .reshape(N_CORES, 128, M)).astype(np.float16)

    # S[l, in_node, out_node] = sum_k W[l, out, k] [idx[l, out, k] == in]
    S = np.zeros((L, M, M), np.float32)
    out_idx = np.broadcast_to(np.arange(M)[None, :, None], (L, M, K))
    lay_idx = np.broadcast_to(np.arange(L)[:, None, None], (L, M, K))
    np.add.at(S, (lay_idx.ravel(), idx.ravel(), out_idx.ravel()), W.ravel())
    # sc[p, ((l*NB + o)*NB + i)*128 + m] = S[l, 128i+p, 128o+m]
    sc = np.ascontiguousarray(
        S.reshape(L, NB, 128, NB, 128).transpose(2, 0, 3, 1, 4)
        .reshape(128, L * SCOLS)).astype(np.float16)

    # bp[m, l*NB + o] = b[l, 128o + m]
    bp = np.ascontiguousarray(
        b.reshape(L, NB, 128).transpose(2, 0, 1).reshape(128, L * NB)
    ).astype(np.float32)

    return xs, sc, bp


def device_args(xs, sc, bp, in_names, zero_outs):
    import jax
    from jax.sharding import Mesh, NamedSharding, PartitionSpec
    mesh = Mesh(np.asarray(jax.devices()[:N_CORES]), ("core",))
    sh_core = NamedSharding(mesh, PartitionSpec("core"))
    sh_rep = NamedSharding(mesh, PartitionSpec())
    per_name = {"xs": (xs.reshape(N_CORES * 128, M), sh_core),
                "sc": (sc, sh_rep), "bp": (bp, sh_rep)}
    args = []
    for n in in_names:
        arr, sh = per_name[n]
        args.append(jax.device_put(arr, sh))
    for z in zero_outs:
        args.append(jax.device_put(
            np.concatenate([z] * N_CORES, axis=0), sh_core))
    return args


def kernel(x, idx, W, b):
    import jax
    x = np.asarray(x, dtype=np.float32)
    idx = np.asarray(idx, dtype=np.int32)
    W = np.asarray(W, dtype=np.float32)
    b = np.asarray(b, dtype=np.float32)

    f, in_names, out_names, zero_outs = _get_runner()
    xs, sc, bp = _prep_inputs(x, idx, W, b)
    args = device_args(xs, sc, bp, in_names, zero_outs)

    outs = f(*args)
    jax.block_until_ready(outs)
    full = np.asarray(outs[0])                       # [8*128, 2048] f16
    # out[128c+bb, 128o+p] = full[c][p, 128o+bb]
    res = full.reshape(N_CORES, 128, NB, BC).transpose(0, 3, 2, 1)
    return np.ascontiguousarray(res.reshape(B, M)).astype(np.float32)


if __name__ == "__main__":
    rng = np.random.default_rng(0)
    x = rng.standard_normal((B, M)).astype(np.float32)
    idx = rng.integers(0, M, size=(L, M, K)).astype(np.int32)
    W = rng.standard_normal((L, M, K)).astype(np.float32)
    b = rng.standard_normal((L, M)).astype(np.float32)
    out = kernel(x=x, idx=idx, W=W, b=b)
    v = 1.0 / (1.0 + np.exp(-(x + 1.0)))
    for l in range(L):
        g = v[:, idx[l]]
        v = 1.0 / (1.0 + np.exp(-(np.einsum('bmk,mk->bm', g, W[l]) + b[l])))
    err = np.abs(out - v).max() / max(np.abs(v).max(), 1e-9)
    print("rel err vs numpy:", err)
